# revision 36
# baseline (speedup 1.0000x reference)
"""Trainium2 Bass kernel for nn_DifferentiableProjector (volume rendering).

Math (per ray i, samples s=0..S-1, channels c):
    T_excl[s] = exp(-DT * sum_{s'<s} rho[s'])
    T_incl[s] = exp(-DT * sum_{s'<=s} rho[s'])
    w[s]      = T_excl[s] - T_incl[s]        (= T_excl * alpha)
    out[i,c]  = sum_s w[s] * f[i,s,c]

Sharding: data-parallel over rays, 65536 rays -> 8 cores x 8192 rays.

Design (all compute in "transposed space", s on partitions):
  - host casts f/rho to fp16; f pre-transposed to [N, C, S] so a single
    xbar-transpose DMA per 512-ray tile yields fT [s, (c, i)] in SBUF
  - cumsum over s (partition axis) via triangular-ones matmuls on TensorE
    (fp32 PSUM); w = exp(excl) - exp(incl) with fp32 exps (fp16 would
    cancel), cast to fp16
  - the big multiply on VectorE at 2x (fp16; broadcast over the OUTER
    free dim keeps innermost step 1)
  - segment-reduce over s: 16 accumulating one-hot matmuls on TensorE
    route channel-c column sums into PSUM row c -> [16, T] per tile
  - ALL xbar-transpose DMAs on the sync queue (concurrent transposes
    from two HWDGE queues corrupt each other); plain copies on scalar
  - output accumulates c-major [C, 8192] in SBUF; one contiguous DMA
"""

import numpy as np

import concourse.bass as bass
import concourse.tile as tile
from concourse.bacc import Bacc
from concourse import mybir
from concourse.bass_utils import run_bass_kernel_spmd

H, W, S, C = 256, 256, 128, 16
N = H * W
NCORES = 8
NS = N // NCORES          # rays per core
P = 128                   # partitions (= S)
T = 512                   # rays per tile
DT = (6.0 - 2.0) / S

_cached = {}

# test-harness hooks (ignored by grading path)
TRACE = False
LAST_RESULTS = None

F16 = mybir.dt.float16
F32 = mybir.dt.float32


def _build_nc(ns: int = NS) -> bass.Bass:
    ntiles = ns // T
    nc = Bacc()
    # host supplies tensors pre-transposed AND pre-tiled:
    #   rho [ntiles*S, T]  (tile t rows t*S:(t+1)*S = rho[s, i] slab)
    #   f   [ntiles*S, C*T] (tile t rows = f[s, (c, i)] slab, contiguous)
    rho_d = nc.dram_tensor("rho", [ntiles * S, T], F16, kind="ExternalInput")
    f_d = nc.dram_tensor("f", [ntiles * S, C * T], F16, kind="ExternalInput")
    cst_d = nc.dram_tensor("consts", [P, 2 * P + C * C], F16, kind="ExternalInput")
    out_d = nc.dram_tensor("out", [C, ns], F32, kind="ExternalOutput")

    with tile.TileContext(nc) as tc:
        with (
            tc.tile_pool(name="cpool", bufs=1) as cpool,
            tc.tile_pool(name="fpool", bufs=5) as fpool,
            tc.tile_pool(name="tpool", bufs=3) as tpool,
            tc.tile_pool(name="spool", bufs=4) as spool,
            tc.tile_pool(name="opool", bufs=1) as opool,
            tc.tile_pool(name="psc", bufs=1, space="PSUM") as psc,
            tc.tile_pool(name="pso", bufs=3, space="PSUM") as pso,
        ):
            consts = cpool.tile([P, 2 * P + C * C], F16)
            nc.scalar.dma_start(out=consts, in_=cst_d[:, :])
            u_excl = consts[:, 0:P]
            u_incl = consts[:, P : 2 * P]
            # E_c = consts[:, 2P + 16c : 2P + 16c + 16]: column m one-hot at c
            e_base = 2 * P

            # persistent per-core output accumulator [C, ns] fp32
            out_acc = opool.tile([C, ns], F32)

            def tile_front(t):
                """DMA loads + w pipeline + big multiply for tile t."""
                r0 = t * T
                # fT[s, c, i]: plain strided DMA (1 KB runs per (s, c)) in 4
                # channel-slabs so the multiply can start on the first slab;
                # alternate queues to keep both DGE streams busy
                fT = fpool.tile([P, C, T], F16, tag="fT")
                f_eng = nc.sync if t % 2 == 0 else nc.scalar
                f_slab = f_d[t * S : (t + 1) * S, :].rearrange(
                    "s (c i) -> s c i", c=C
                )
                nsplit = 4 if t < 2 else 1
                for q in range(nsplit):
                    c0 = q * (C // nsplit)
                    f_eng.dma_start(
                        out=fT[:, c0 : c0 + C // nsplit, :],
                        in_=f_slab[:, c0 : c0 + C // nsplit, :],
                    )
                # rhoT[s, i]
                rhoT = spool.tile([P, T], F16, tag="rhoT")
                nc.sync.dma_start(out=rhoT, in_=rho_d[t * S : (t + 1) * S, :])

                # cumsum over s (partition axis) via triangular matmuls
                pexc = psc.tile([P, T], F32, tag="pexc")
                pinc = psc.tile([P, T], F32, tag="pinc")
                nc.tensor.matmul(pexc, u_excl, rhoT, start=True, stop=True)
                nc.tensor.matmul(pinc, u_incl, rhoT, start=True, stop=True)

                # exps in fp32 (w = e1 - e2 cancels; fp16 here costs ~4% on w)
                e1 = spool.tile([P, T], F32, tag="e1")
                e2 = spool.tile([P, T], F32, tag="e2")
                nc.scalar.activation(
                    e1, pexc, mybir.ActivationFunctionType.Exp, scale=-DT
                )
                nc.scalar.activation(
                    e2, pinc, mybir.ActivationFunctionType.Exp, scale=-DT
                )
                w = spool.tile([P, T], F16, tag="w")
                nc.vector.tensor_sub(w, e1, e2)

                # tmp[s, c, i] = fT[s, c, i] * w[s, i], per loaded slab
                tmp = tpool.tile([P, C, T], F16, tag="tmp")
                for q in range(nsplit):
                    c0 = q * (C // nsplit)
                    nc.vector.tensor_mul(
                        tmp[:, c0 : c0 + C // nsplit, :],
                        fT[:, c0 : c0 + C // nsplit, :],
                        w[:, None, :].broadcast_to((P, C // nsplit, T)),
                    )
                return tmp

            def tile_back(t, tmp_pair):
                """Reduce + evacuate for a pair of tiles, sharing each E_c
                weight load across both tiles' matmuls."""
                psums = [pso.tile([C, T], F32, tag=f"po{i}", name=f"po{i}_{t}") for i in range(2)]
                for c in range(C):
                    lhs = consts[:, e_base + c * C : e_base + (c + 1) * C]
                    for tmp_t, po in zip(tmp_pair, psums):
                        nc.tensor.matmul(
                            po,
                            lhs,
                            tmp_t[:, c, :],
                            start=(c == 0),
                            stop=(c == C - 1),
                        )
                for k, po in enumerate(psums):
                    r0 = (t + k) * T
                    nc.scalar.activation(
                        out_acc[:, r0 : r0 + T],
                        po,
                        mybir.ActivationFunctionType.Copy,
                    )
                # stream this pair's output out now (overlaps later tiles)
                nc.scalar.dma_start(
                    out=out_d[:, t * T : (t + 2) * T],
                    in_=out_acc[:, t * T : (t + 2) * T],
                )

            for t in range(0, ntiles, 2):
                tmp_a = tile_front(t)
                tmp_b = tile_front(t + 1)
                tile_back(t, [tmp_a, tmp_b])
    if not nc.is_finalized():
        nc.finalize()
    return nc


def _consts() -> np.ndarray:
    u_excl = np.triu(np.ones((P, P), np.float16), 1)
    u_incl = np.triu(np.ones((P, P), np.float16), 0)
    # E[:, c*C + m] = 1 if m == c else 0  (all rows identical)
    e = np.tile(np.eye(C, dtype=np.float16).reshape(1, C * C), (P, 1))
    return np.ascontiguousarray(np.concatenate([u_excl, u_incl, e], axis=1))


def kernel(rho: np.ndarray, f: np.ndarray) -> np.ndarray:
    global LAST_RESULTS
    if "nc" not in _cached:
        _cached["nc"] = _build_nc()
        _cached["consts"] = _consts()
    nc = _cached["nc"]

    rho16 = np.asarray(rho, dtype=np.float16).reshape(N, S)
    f16 = np.asarray(f, dtype=np.float16).reshape(N, S, C)
    cst = _cached["consts"]
    ntiles = NS // T

    in_maps = []
    for i in range(NCORES):
        sl = slice(i * NS, (i + 1) * NS)
        # [rays, S] -> [ntiles, S, T] / [rays, S, C] -> [ntiles, S, C, T]
        rho_t = np.ascontiguousarray(
            rho16[sl].reshape(ntiles, T, S).transpose(0, 2, 1)
        ).reshape(ntiles * S, T)
        f_t = np.ascontiguousarray(
            f16[sl].reshape(ntiles, T, S, C).transpose(0, 2, 3, 1)
        ).reshape(ntiles * S, C * T)
        in_maps.append({"rho": rho_t, "f": f_t, "consts": cst})
    res = run_bass_kernel_spmd(nc, in_maps, list(range(NCORES)), trace=TRACE)
    LAST_RESULTS = res
    out = np.concatenate(
        [res.results[i]["out"] for i in range(NCORES)], axis=1
    )  # [C, N]
    return out.reshape(C, H, W)[None].astype(np.float32, copy=False)


# revision 37
# speedup vs baseline: 1.1509x; 1.1509x over previous
"""Trainium2 Bass kernel for nn_DifferentiableProjector (volume rendering).

Math (per ray i, samples s=0..S-1, channels c):
    T_excl[s] = exp(-DT * sum_{s'<s} rho[s'])
    T_incl[s] = exp(-DT * sum_{s'<=s} rho[s'])
    w[s]      = T_excl[s] - T_incl[s]        (= T_excl * alpha)
    out[i,c]  = sum_s w[s] * f[i,s,c]

Sharding: data-parallel over rays, 65536 rays -> 8 cores x 8192 rays.

Design (all compute in "transposed space", s on partitions):
  - host casts f/rho to fp16; f pre-transposed to [N, C, S] so a single
    xbar-transpose DMA per 512-ray tile yields fT [s, (c, i)] in SBUF
  - cumsum over s (partition axis) via triangular-ones matmuls on TensorE
    (fp32 PSUM); w = exp(excl) - exp(incl) with fp32 exps (fp16 would
    cancel), cast to fp16
  - the big multiply on VectorE at 2x (fp16; broadcast over the OUTER
    free dim keeps innermost step 1)
  - segment-reduce over s: 16 accumulating one-hot matmuls on TensorE
    route channel-c column sums into PSUM row c -> [16, T] per tile
  - ALL xbar-transpose DMAs on the sync queue (concurrent transposes
    from two HWDGE queues corrupt each other); plain copies on scalar
  - output accumulates c-major [C, 8192] in SBUF; one contiguous DMA
"""

import numpy as np

import concourse.bass as bass
import concourse.tile as tile
from concourse.bacc import Bacc
from concourse import mybir
from concourse.bass_utils import run_bass_kernel_spmd

H, W, S, C = 256, 256, 128, 16
N = H * W
NCORES = 8
NS = N // NCORES          # rays per core
P = 128                   # partitions (= S)
T = 512                   # rays per tile
DT = (6.0 - 2.0) / S

_cached = {}

# test-harness hooks (ignored by grading path)
TRACE = False
LAST_RESULTS = None

F16 = mybir.dt.float16
F32 = mybir.dt.float32


def _build_nc(ns: int = NS) -> bass.Bass:
    ntiles = ns // T
    nc = Bacc()
    # host supplies tensors pre-transposed AND pre-tiled:
    #   rho [ntiles*S, T]  (tile t rows t*S:(t+1)*S = rho[s, i] slab)
    #   f   [ntiles*S, C*T] (tile t rows = f[s, (c, i)] slab, contiguous)
    rho_d = nc.dram_tensor("rho", [ntiles * S, T], F16, kind="ExternalInput")
    f_d = nc.dram_tensor("f", [ntiles * S, C * T], F16, kind="ExternalInput")
    cst_d = nc.dram_tensor("consts", [P, 2 * P + C * C], F16, kind="ExternalInput")
    out_d = nc.dram_tensor("out", [C, ns], F32, kind="ExternalOutput")

    with tile.TileContext(nc) as tc:
        with (
            tc.tile_pool(name="cpool", bufs=1) as cpool,
            tc.tile_pool(name="fpool", bufs=5) as fpool,
            tc.tile_pool(name="tpool", bufs=3) as tpool,
            tc.tile_pool(name="spool", bufs=4) as spool,
            tc.tile_pool(name="opool", bufs=1) as opool,
            tc.tile_pool(name="psc", bufs=2, space="PSUM") as psc,
            tc.tile_pool(name="pso", bufs=2, space="PSUM") as pso,
        ):
            consts = cpool.tile([P, 2 * P + C * C], F16)
            nc.scalar.dma_start(out=consts, in_=cst_d[:, :])
            u_excl = consts[:, 0:P]
            u_incl = consts[:, P : 2 * P]
            # E_c = consts[:, 2P + 16c : 2P + 16c + 16]: column m one-hot at c
            e_base = 2 * P

            # persistent per-core output accumulator [C, ns] fp32
            out_acc = opool.tile([C, ns], F32)

            def tile_front(t):
                """DMA loads + w pipeline + big multiply for tile t."""
                r0 = t * T
                # fT[s, c, i]: plain strided DMA (1 KB runs per (s, c)) in 4
                # channel-slabs so the multiply can start on the first slab;
                # alternate queues to keep both DGE streams busy
                fT = fpool.tile([P, C, T], F16, tag="fT")
                f_eng = nc.sync if t % 2 == 0 else nc.scalar
                f_slab = f_d[t * S : (t + 1) * S, :].rearrange(
                    "s (c i) -> s c i", c=C
                )
                nsplit = 4 if t < 2 else 1
                for q in range(nsplit):
                    c0 = q * (C // nsplit)
                    f_eng.dma_start(
                        out=fT[:, c0 : c0 + C // nsplit, :],
                        in_=f_slab[:, c0 : c0 + C // nsplit, :],
                    )
                # rhoT[s, i]
                rhoT = spool.tile([P, T], F16, tag="rhoT")
                nc.sync.dma_start(out=rhoT, in_=rho_d[t * S : (t + 1) * S, :])

                # cumsum over s (partition axis) via triangular matmuls
                pexc = psc.tile([P, T], F32, tag="pexc")
                pinc = psc.tile([P, T], F32, tag="pinc")
                nc.tensor.matmul(pexc, u_excl, rhoT, start=True, stop=True)
                nc.tensor.matmul(pinc, u_incl, rhoT, start=True, stop=True)

                # exps in fp32 (w = e1 - e2 cancels; fp16 here costs ~4% on w)
                e1 = spool.tile([P, T], F32, tag="e1")
                e2 = spool.tile([P, T], F32, tag="e2")
                nc.scalar.activation(
                    e1, pexc, mybir.ActivationFunctionType.Exp, scale=-DT
                )
                nc.scalar.activation(
                    e2, pinc, mybir.ActivationFunctionType.Exp, scale=-DT
                )
                w = spool.tile([P, T], F16, tag="w")
                nc.vector.tensor_sub(w, e1, e2)

                # tmp[s, c, i] = fT[s, c, i] * w[s, i], per loaded slab
                tmp = tpool.tile([P, C, T], F16, tag="tmp")
                for q in range(nsplit):
                    c0 = q * (C // nsplit)
                    nc.vector.tensor_mul(
                        tmp[:, c0 : c0 + C // nsplit, :],
                        fT[:, c0 : c0 + C // nsplit, :],
                        w[:, None, :].broadcast_to((P, C // nsplit, T)),
                    )
                return tmp

            def tile_back(t, tmp_pair):
                """Reduce + evacuate for a pair of tiles, sharing each E_c
                weight load across both tiles' matmuls."""
                psums = [pso.tile([C, T], F32, tag=f"po{i}", name=f"po{i}_{t}") for i in range(2)]
                for c in range(C):
                    lhs = consts[:, e_base + c * C : e_base + (c + 1) * C]
                    for tmp_t, po in zip(tmp_pair, psums):
                        nc.tensor.matmul(
                            po,
                            lhs,
                            tmp_t[:, c, :],
                            start=(c == 0),
                            stop=(c == C - 1),
                        )
                for k, po in enumerate(psums):
                    r0 = (t + k) * T
                    nc.scalar.activation(
                        out_acc[:, r0 : r0 + T],
                        po,
                        mybir.ActivationFunctionType.Copy,
                    )
                # stream this pair's output out now (overlaps later tiles)
                nc.scalar.dma_start(
                    out=out_d[:, t * T : (t + 2) * T],
                    in_=out_acc[:, t * T : (t + 2) * T],
                )

            for t in range(0, ntiles, 2):
                tmp_a = tile_front(t)
                tmp_b = tile_front(t + 1)
                tile_back(t, [tmp_a, tmp_b])
    if not nc.is_finalized():
        nc.finalize()
    return nc


def _consts() -> np.ndarray:
    u_excl = np.triu(np.ones((P, P), np.float16), 1)
    u_incl = np.triu(np.ones((P, P), np.float16), 0)
    # E[:, c*C + m] = 1 if m == c else 0  (all rows identical)
    e = np.tile(np.eye(C, dtype=np.float16).reshape(1, C * C), (P, 1))
    return np.ascontiguousarray(np.concatenate([u_excl, u_incl, e], axis=1))


def kernel(rho: np.ndarray, f: np.ndarray) -> np.ndarray:
    global LAST_RESULTS
    if "nc" not in _cached:
        _cached["nc"] = _build_nc()
        _cached["consts"] = _consts()
    nc = _cached["nc"]

    rho16 = np.asarray(rho, dtype=np.float16).reshape(N, S)
    f16 = np.asarray(f, dtype=np.float16).reshape(N, S, C)
    cst = _cached["consts"]
    ntiles = NS // T

    in_maps = []
    for i in range(NCORES):
        sl = slice(i * NS, (i + 1) * NS)
        # [rays, S] -> [ntiles, S, T] / [rays, S, C] -> [ntiles, S, C, T]
        rho_t = np.ascontiguousarray(
            rho16[sl].reshape(ntiles, T, S).transpose(0, 2, 1)
        ).reshape(ntiles * S, T)
        f_t = np.ascontiguousarray(
            f16[sl].reshape(ntiles, T, S, C).transpose(0, 2, 3, 1)
        ).reshape(ntiles * S, C * T)
        in_maps.append({"rho": rho_t, "f": f_t, "consts": cst})
    res = run_bass_kernel_spmd(nc, in_maps, list(range(NCORES)), trace=TRACE)
    LAST_RESULTS = res
    out = np.concatenate(
        [res.results[i]["out"] for i in range(NCORES)], axis=1
    )  # [C, N]
    return out.reshape(C, H, W)[None].astype(np.float32, copy=False)


# revision 38
# speedup vs baseline: 1.1519x; 1.0009x over previous
"""Trainium2 Bass kernel for nn_DifferentiableProjector (volume rendering).

Math (per ray i, samples s=0..S-1, channels c):
    T_excl[s] = exp(-DT * sum_{s'<s} rho[s'])
    T_incl[s] = exp(-DT * sum_{s'<=s} rho[s'])
    w[s]      = T_excl[s] - T_incl[s]        (= T_excl * alpha)
    out[i,c]  = sum_s w[s] * f[i,s,c]

Sharding: data-parallel over rays, 65536 rays -> 8 cores x 8192 rays.

Design (all compute in "transposed space", s on partitions):
  - host casts f/rho to fp16; f pre-transposed to [N, C, S] so a single
    xbar-transpose DMA per 512-ray tile yields fT [s, (c, i)] in SBUF
  - cumsum over s (partition axis) via triangular-ones matmuls on TensorE
    (fp32 PSUM); w = exp(excl) - exp(incl) with fp32 exps (fp16 would
    cancel), cast to fp16
  - the big multiply on VectorE at 2x (fp16; broadcast over the OUTER
    free dim keeps innermost step 1)
  - segment-reduce over s: 16 accumulating one-hot matmuls on TensorE
    route channel-c column sums into PSUM row c -> [16, T] per tile
  - ALL xbar-transpose DMAs on the sync queue (concurrent transposes
    from two HWDGE queues corrupt each other); plain copies on scalar
  - output accumulates c-major [C, 8192] in SBUF; one contiguous DMA
"""

import numpy as np

import concourse.bass as bass
import concourse.tile as tile
from concourse.bacc import Bacc
from concourse import mybir
from concourse.bass_utils import run_bass_kernel_spmd

H, W, S, C = 256, 256, 128, 16
N = H * W
NCORES = 8
NS = N // NCORES          # rays per core
P = 128                   # partitions (= S)
T = 512                   # rays per tile
DT = (6.0 - 2.0) / S

_cached = {}

# test-harness hooks (ignored by grading path)
TRACE = False
LAST_RESULTS = None

F16 = mybir.dt.float16
F32 = mybir.dt.float32


def _build_nc(ns: int = NS) -> bass.Bass:
    ntiles = ns // T
    nc = Bacc()
    # host supplies tensors pre-transposed AND pre-tiled:
    #   rho [ntiles*S, T]  (tile t rows t*S:(t+1)*S = rho[s, i] slab)
    #   f   [ntiles*S, C*T] (tile t rows = f[s, (c, i)] slab, contiguous)
    rho_d = nc.dram_tensor("rho", [ntiles * S, T], F16, kind="ExternalInput")
    f_d = nc.dram_tensor("f", [ntiles * S, C * T], F16, kind="ExternalInput")
    cst_d = nc.dram_tensor("consts", [P, 2 * P + C * C], F16, kind="ExternalInput")
    out_d = nc.dram_tensor("out", [C, ns], F32, kind="ExternalOutput")

    with tile.TileContext(nc) as tc:
        with (
            tc.tile_pool(name="cpool", bufs=1) as cpool,
            tc.tile_pool(name="fpool", bufs=5) as fpool,
            tc.tile_pool(name="tpool", bufs=3) as tpool,
            tc.tile_pool(name="spool", bufs=4) as spool,
            tc.tile_pool(name="opool", bufs=1) as opool,
            tc.tile_pool(name="psc", bufs=2, space="PSUM") as psc,
            tc.tile_pool(name="pso", bufs=2, space="PSUM") as pso,
        ):
            consts = cpool.tile([P, 2 * P + C * C], F16)
            nc.scalar.dma_start(out=consts, in_=cst_d[:, :])
            u_excl = consts[:, 0:P]
            u_incl = consts[:, P : 2 * P]
            # E_c = consts[:, 2P + 16c : 2P + 16c + 16]: column m one-hot at c
            e_base = 2 * P

            # persistent per-core output accumulator [C, ns] fp32
            out_acc = opool.tile([C, ns], F32)

            # all rho slabs upfront: [s, (tile, i)] fp16 = 16 KB/partition
            rho_all = cpool.tile([P, ntiles, T], F16)
            nc.sync.dma_start(
                out=rho_all,
                in_=rho_d[:, :].rearrange("(t s) i -> s t i", s=S),
            )

            def tile_front(t):
                """DMA loads + w pipeline + big multiply for tile t."""
                r0 = t * T
                # fT[s, c, i]: plain strided DMA (1 KB runs per (s, c)) in 4
                # channel-slabs so the multiply can start on the first slab;
                # alternate queues to keep both DGE streams busy
                fT = fpool.tile([P, C, T], F16, tag="fT")
                f_eng = nc.sync if t % 2 == 0 else nc.scalar
                f_slab = f_d[t * S : (t + 1) * S, :].rearrange(
                    "s (c i) -> s c i", c=C
                )
                nsplit = 4 if t < 2 else 1
                for q in range(nsplit):
                    c0 = q * (C // nsplit)
                    f_eng.dma_start(
                        out=fT[:, c0 : c0 + C // nsplit, :],
                        in_=f_slab[:, c0 : c0 + C // nsplit, :],
                    )
                rhoT = rho_all[:, t, :]

                # cumsum over s (partition axis) via triangular matmuls
                pexc = psc.tile([P, T], F32, tag="pexc")
                pinc = psc.tile([P, T], F32, tag="pinc")
                nc.tensor.matmul(pexc, u_excl, rhoT, start=True, stop=True)
                nc.tensor.matmul(pinc, u_incl, rhoT, start=True, stop=True)

                # exps in fp32 (w = e1 - e2 cancels; fp16 here costs ~4% on w)
                e1 = spool.tile([P, T], F32, tag="e1")
                e2 = spool.tile([P, T], F32, tag="e2")
                nc.scalar.activation(
                    e1, pexc, mybir.ActivationFunctionType.Exp, scale=-DT
                )
                nc.scalar.activation(
                    e2, pinc, mybir.ActivationFunctionType.Exp, scale=-DT
                )
                w = spool.tile([P, T], F16, tag="w")
                nc.vector.tensor_sub(w, e1, e2)

                # tmp[s, c, i] = fT[s, c, i] * w[s, i], per loaded slab
                tmp = tpool.tile([P, C, T], F16, tag="tmp")
                for q in range(nsplit):
                    c0 = q * (C // nsplit)
                    nc.vector.tensor_mul(
                        tmp[:, c0 : c0 + C // nsplit, :],
                        fT[:, c0 : c0 + C // nsplit, :],
                        w[:, None, :].broadcast_to((P, C // nsplit, T)),
                    )
                return tmp

            def tile_back(t, tmp_pair):
                """Reduce + evacuate for a pair of tiles, sharing each E_c
                weight load across both tiles' matmuls."""
                psums = [pso.tile([C, T], F32, tag=f"po{i}", name=f"po{i}_{t}") for i in range(2)]
                for c in range(C):
                    lhs = consts[:, e_base + c * C : e_base + (c + 1) * C]
                    for tmp_t, po in zip(tmp_pair, psums):
                        nc.tensor.matmul(
                            po,
                            lhs,
                            tmp_t[:, c, :],
                            start=(c == 0),
                            stop=(c == C - 1),
                        )
                for k, po in enumerate(psums):
                    r0 = (t + k) * T
                    nc.scalar.activation(
                        out_acc[:, r0 : r0 + T],
                        po,
                        mybir.ActivationFunctionType.Copy,
                    )
                # stream this pair's output out now (overlaps later tiles)
                nc.scalar.dma_start(
                    out=out_d[:, t * T : (t + 2) * T],
                    in_=out_acc[:, t * T : (t + 2) * T],
                )

            for t in range(0, ntiles, 2):
                tmp_a = tile_front(t)
                tmp_b = tile_front(t + 1)
                tile_back(t, [tmp_a, tmp_b])
    if not nc.is_finalized():
        nc.finalize()
    return nc


def _consts() -> np.ndarray:
    u_excl = np.triu(np.ones((P, P), np.float16), 1)
    u_incl = np.triu(np.ones((P, P), np.float16), 0)
    # E[:, c*C + m] = 1 if m == c else 0  (all rows identical)
    e = np.tile(np.eye(C, dtype=np.float16).reshape(1, C * C), (P, 1))
    return np.ascontiguousarray(np.concatenate([u_excl, u_incl, e], axis=1))


def kernel(rho: np.ndarray, f: np.ndarray) -> np.ndarray:
    global LAST_RESULTS
    if "nc" not in _cached:
        _cached["nc"] = _build_nc()
        _cached["consts"] = _consts()
    nc = _cached["nc"]

    rho16 = np.asarray(rho, dtype=np.float16).reshape(N, S)
    f16 = np.asarray(f, dtype=np.float16).reshape(N, S, C)
    cst = _cached["consts"]
    ntiles = NS // T

    in_maps = []
    for i in range(NCORES):
        sl = slice(i * NS, (i + 1) * NS)
        # [rays, S] -> [ntiles, S, T] / [rays, S, C] -> [ntiles, S, C, T]
        rho_t = np.ascontiguousarray(
            rho16[sl].reshape(ntiles, T, S).transpose(0, 2, 1)
        ).reshape(ntiles * S, T)
        f_t = np.ascontiguousarray(
            f16[sl].reshape(ntiles, T, S, C).transpose(0, 2, 3, 1)
        ).reshape(ntiles * S, C * T)
        in_maps.append({"rho": rho_t, "f": f_t, "consts": cst})
    res = run_bass_kernel_spmd(nc, in_maps, list(range(NCORES)), trace=TRACE)
    LAST_RESULTS = res
    out = np.concatenate(
        [res.results[i]["out"] for i in range(NCORES)], axis=1
    )  # [C, N]
    return out.reshape(C, H, W)[None].astype(np.float32, copy=False)


# revision 39
# speedup vs baseline: 1.1623x; 1.0090x over previous
"""Trainium2 Bass kernel for nn_DifferentiableProjector (volume rendering).

Math (per ray i, samples s=0..S-1, channels c):
    T_excl[s] = exp(-DT * sum_{s'<s} rho[s'])
    T_incl[s] = exp(-DT * sum_{s'<=s} rho[s'])
    w[s]      = T_excl[s] - T_incl[s]        (= T_excl * alpha)
    out[i,c]  = sum_s w[s] * f[i,s,c]

Sharding: data-parallel over rays, 65536 rays -> 8 cores x 8192 rays.

Design (all compute in "transposed space", s on partitions):
  - host casts f/rho to fp16; f pre-transposed to [N, C, S] so a single
    xbar-transpose DMA per 512-ray tile yields fT [s, (c, i)] in SBUF
  - cumsum over s (partition axis) via triangular-ones matmuls on TensorE
    (fp32 PSUM); w = exp(excl) - exp(incl) with fp32 exps (fp16 would
    cancel), cast to fp16
  - the big multiply on VectorE at 2x (fp16; broadcast over the OUTER
    free dim keeps innermost step 1)
  - segment-reduce over s: 16 accumulating one-hot matmuls on TensorE
    route channel-c column sums into PSUM row c -> [16, T] per tile
  - ALL xbar-transpose DMAs on the sync queue (concurrent transposes
    from two HWDGE queues corrupt each other); plain copies on scalar
  - output accumulates c-major [C, 8192] in SBUF; one contiguous DMA
"""

import numpy as np

import concourse.bass as bass
import concourse.tile as tile
from concourse.bacc import Bacc
from concourse import mybir
from concourse.bass_utils import run_bass_kernel_spmd

H, W, S, C = 256, 256, 128, 16
N = H * W
NCORES = 8
NS = N // NCORES          # rays per core
P = 128                   # partitions (= S)
T = 512                   # rays per tile
DT = (6.0 - 2.0) / S

_cached = {}

# test-harness hooks (ignored by grading path)
TRACE = False
LAST_RESULTS = None

F16 = mybir.dt.float16
F32 = mybir.dt.float32


def _build_nc(ns: int = NS) -> bass.Bass:
    ntiles = ns // T
    nc = Bacc()
    # host supplies tensors pre-transposed AND pre-tiled:
    #   rho [ntiles*S, T]  (tile t rows t*S:(t+1)*S = rho[s, i] slab)
    #   f   [ntiles*S, C*T] (tile t rows = f[s, (c, i)] slab, contiguous)
    rho_d = nc.dram_tensor("rho", [ntiles * S, T], F16, kind="ExternalInput")
    f_d = nc.dram_tensor("f", [ntiles * S, C * T], F16, kind="ExternalInput")
    cst_d = nc.dram_tensor("consts", [P, 2 * P + C * C], F16, kind="ExternalInput")
    out_d = nc.dram_tensor("out", [C, ns], F32, kind="ExternalOutput")

    with tile.TileContext(nc) as tc:
        with (
            tc.tile_pool(name="cpool", bufs=1) as cpool,
            tc.tile_pool(name="fpool", bufs=6) as fpool,
            tc.tile_pool(name="tpool", bufs=3) as tpool,
            tc.tile_pool(name="spool", bufs=4) as spool,
            tc.tile_pool(name="opool", bufs=3) as opool,
            tc.tile_pool(name="psc", bufs=2, space="PSUM") as psc,
            tc.tile_pool(name="pso", bufs=2, space="PSUM") as pso,
        ):
            consts = cpool.tile([P, 2 * P + C * C], F16)
            nc.scalar.dma_start(out=consts, in_=cst_d[:, :])
            u_excl = consts[:, 0:P]
            u_incl = consts[:, P : 2 * P]
            # E_c = consts[:, 2P + 16c : 2P + 16c + 16]: column m one-hot at c
            e_base = 2 * P


            # all rho slabs upfront: [s, (tile, i)] fp16 = 16 KB/partition
            rho_all = cpool.tile([P, ntiles, T], F16)
            nc.sync.dma_start(
                out=rho_all,
                in_=rho_d[:, :].rearrange("(t s) i -> s t i", s=S),
            )

            def tile_front(t):
                """DMA loads + w pipeline + big multiply for tile t."""
                r0 = t * T
                # fT[s, c, i]: plain strided DMA (1 KB runs per (s, c)) in 4
                # channel-slabs so the multiply can start on the first slab;
                # alternate queues to keep both DGE streams busy
                fT = fpool.tile([P, C, T], F16, tag="fT")
                f_eng = nc.sync if t % 2 == 0 else nc.scalar
                f_slab = f_d[t * S : (t + 1) * S, :].rearrange(
                    "s (c i) -> s c i", c=C
                )
                nsplit = 4 if t < 2 else 1
                for q in range(nsplit):
                    c0 = q * (C // nsplit)
                    f_eng.dma_start(
                        out=fT[:, c0 : c0 + C // nsplit, :],
                        in_=f_slab[:, c0 : c0 + C // nsplit, :],
                    )
                rhoT = rho_all[:, t, :]

                # cumsum over s (partition axis) via triangular matmuls
                pexc = psc.tile([P, T], F32, tag="pexc")
                pinc = psc.tile([P, T], F32, tag="pinc")
                nc.tensor.matmul(pexc, u_excl, rhoT, start=True, stop=True)
                nc.tensor.matmul(pinc, u_incl, rhoT, start=True, stop=True)

                # exps in fp32 (w = e1 - e2 cancels; fp16 here costs ~4% on w)
                e1 = spool.tile([P, T], F32, tag="e1")
                e2 = spool.tile([P, T], F32, tag="e2")
                nc.scalar.activation(
                    e1, pexc, mybir.ActivationFunctionType.Exp, scale=-DT
                )
                nc.scalar.activation(
                    e2, pinc, mybir.ActivationFunctionType.Exp, scale=-DT
                )
                w = spool.tile([P, T], F16, tag="w")
                nc.vector.tensor_sub(w, e1, e2)

                # tmp[s, c, i] = fT[s, c, i] * w[s, i], per loaded slab
                tmp = tpool.tile([P, C, T], F16, tag="tmp")
                for q in range(nsplit):
                    c0 = q * (C // nsplit)
                    nc.vector.tensor_mul(
                        tmp[:, c0 : c0 + C // nsplit, :],
                        fT[:, c0 : c0 + C // nsplit, :],
                        w[:, None, :].broadcast_to((P, C // nsplit, T)),
                    )
                return tmp

            def tile_back(t, tmp_pair):
                """Reduce + evacuate for a pair of tiles, sharing each E_c
                weight load across both tiles' matmuls."""
                psums = [pso.tile([C, T], F32, tag=f"po{i}", name=f"po{i}_{t}") for i in range(2)]
                for c in range(C):
                    lhs = consts[:, e_base + c * C : e_base + (c + 1) * C]
                    for tmp_t, po in zip(tmp_pair, psums):
                        nc.tensor.matmul(
                            po,
                            lhs,
                            tmp_t[:, c, :],
                            start=(c == 0),
                            stop=(c == C - 1),
                        )
                out_pair = opool.tile([C, 2 * T], F32, tag="out_pair",
                                      name=f"out_pair_{t}")
                for k, po in enumerate(psums):
                    nc.scalar.activation(
                        out_pair[:, k * T : (k + 1) * T],
                        po,
                        mybir.ActivationFunctionType.Copy,
                    )
                # stream this pair's output out now (overlaps later tiles)
                nc.scalar.dma_start(
                    out=out_d[:, t * T : (t + 2) * T],
                    in_=out_pair,
                )

            for t in range(0, ntiles, 2):
                tmp_a = tile_front(t)
                tmp_b = tile_front(t + 1)
                tile_back(t, [tmp_a, tmp_b])
    if not nc.is_finalized():
        nc.finalize()
    return nc


def _consts() -> np.ndarray:
    u_excl = np.triu(np.ones((P, P), np.float16), 1)
    u_incl = np.triu(np.ones((P, P), np.float16), 0)
    # E[:, c*C + m] = 1 if m == c else 0  (all rows identical)
    e = np.tile(np.eye(C, dtype=np.float16).reshape(1, C * C), (P, 1))
    return np.ascontiguousarray(np.concatenate([u_excl, u_incl, e], axis=1))


def kernel(rho: np.ndarray, f: np.ndarray) -> np.ndarray:
    global LAST_RESULTS
    if "nc" not in _cached:
        _cached["nc"] = _build_nc()
        _cached["consts"] = _consts()
    nc = _cached["nc"]

    rho16 = np.asarray(rho, dtype=np.float16).reshape(N, S)
    f16 = np.asarray(f, dtype=np.float16).reshape(N, S, C)
    cst = _cached["consts"]
    ntiles = NS // T

    in_maps = []
    for i in range(NCORES):
        sl = slice(i * NS, (i + 1) * NS)
        # [rays, S] -> [ntiles, S, T] / [rays, S, C] -> [ntiles, S, C, T]
        rho_t = np.ascontiguousarray(
            rho16[sl].reshape(ntiles, T, S).transpose(0, 2, 1)
        ).reshape(ntiles * S, T)
        f_t = np.ascontiguousarray(
            f16[sl].reshape(ntiles, T, S, C).transpose(0, 2, 3, 1)
        ).reshape(ntiles * S, C * T)
        in_maps.append({"rho": rho_t, "f": f_t, "consts": cst})
    res = run_bass_kernel_spmd(nc, in_maps, list(range(NCORES)), trace=TRACE)
    LAST_RESULTS = res
    out = np.concatenate(
        [res.results[i]["out"] for i in range(NCORES)], axis=1
    )  # [C, N]
    return out.reshape(C, H, W)[None].astype(np.float32, copy=False)


# revision 40
# speedup vs baseline: 1.1813x; 1.0164x over previous
"""Trainium2 Bass kernel for nn_DifferentiableProjector (volume rendering).

Math (per ray i, samples s=0..S-1, channels c):
    T_excl[s] = exp(-DT * sum_{s'<s} rho[s'])
    T_incl[s] = exp(-DT * sum_{s'<=s} rho[s'])
    w[s]      = T_excl[s] - T_incl[s]        (= T_excl * alpha)
    out[i,c]  = sum_s w[s] * f[i,s,c]

Sharding: data-parallel over rays, 65536 rays -> 8 cores x 8192 rays.

Design (all compute in "transposed space", s on partitions):
  - host casts rho/f to fp16 AND pre-tiles them so each 512-ray tile is a
    fully contiguous [S, C, T] DRAM slab (16 KB/partition rows) -> plain
    max-bandwidth DMAs, alternating the two HWDGE queues (sync/scalar)
  - cumsum over s (the partition axis) via triangular-ones matmuls on
    TensorE (fp32 PSUM); w = exp(-DT*excl) - exp(-DT*incl) with fp32 exps
    (fp16 exps would cancel catastrophically), cast to fp16
  - the big multiply runs on VectorE at 2x (fp16; the w broadcast is over
    the OUTER free dim so the innermost step stays 1)
  - segment-reduce over s: 16 accumulating one-hot matmuls on TensorE
    route channel-c column sums into PSUM row c -> [16, T] per tile;
    E_c weight loads are shared across tile pairs
  - ScalarE evacuates PSUM -> SBUF; outputs stream out per tile-pair,
    c-major [C, rays] = exactly the final [1,C,H,W] layout

Measured on 8 axon trn2 cores: ~129 us HW exec (v1 naive DVE: 415 us);
fp16 DMA roofline ~96 us + ~22 us fixed startup/tail. Rel err ~4e-4.
"""

import numpy as np

import concourse.bass as bass
import concourse.tile as tile
from concourse.bacc import Bacc
from concourse import mybir
from concourse.bass_utils import run_bass_kernel_spmd

H, W, S, C = 256, 256, 128, 16
N = H * W
NCORES = 8
NS = N // NCORES          # rays per core
P = 128                   # partitions (= S)
T = 512                   # rays per tile
DT = (6.0 - 2.0) / S

_cached = {}

# test-harness hooks (ignored by grading path)
TRACE = False
LAST_RESULTS = None

F16 = mybir.dt.float16
F32 = mybir.dt.float32


def _build_nc(ns: int = NS) -> bass.Bass:
    ntiles = ns // T
    nc = Bacc()
    # host supplies tensors pre-transposed AND pre-tiled:
    #   rho [ntiles*S, T]  (tile t rows t*S:(t+1)*S = rho[s, i] slab)
    #   f   [ntiles*S, C*T] (tile t rows = f[s, (c, i)] slab, contiguous)
    rho_d = nc.dram_tensor("rho", [ntiles * S, T], F16, kind="ExternalInput")
    f_d = nc.dram_tensor("f", [ntiles * S, C * T], F16, kind="ExternalInput")
    cst_d = nc.dram_tensor("consts", [P, 2 * P + C * C], F16, kind="ExternalInput")
    out_d = nc.dram_tensor("out", [C, ns], F32, kind="ExternalOutput")

    with tile.TileContext(nc) as tc:
        with (
            tc.tile_pool(name="cpool", bufs=1) as cpool,
            tc.tile_pool(name="fpool", bufs=6) as fpool,
            tc.tile_pool(name="tpool", bufs=3) as tpool,
            tc.tile_pool(name="spool", bufs=4) as spool,
            tc.tile_pool(name="opool", bufs=3) as opool,
            tc.tile_pool(name="psc", bufs=2, space="PSUM") as psc,
            tc.tile_pool(name="pso", bufs=2, space="PSUM") as pso,
        ):
            consts = cpool.tile([P, 2 * P + C * C], F16)
            nc.scalar.dma_start(out=consts, in_=cst_d[:, :])
            u_excl = consts[:, 0:P]
            u_incl = consts[:, P : 2 * P]
            # E_c = consts[:, 2P + 16c : 2P + 16c + 16]: column m one-hot at c
            e_base = 2 * P


            # all rho slabs upfront: [s, (tile, i)] fp16 = 16 KB/partition
            rho_all = cpool.tile([P, ntiles, T], F16)
            nc.sync.dma_start(
                out=rho_all,
                in_=rho_d[:, :].rearrange("(t s) i -> s t i", s=S),
            )

            def tile_front(t):
                """DMA loads + w pipeline + big multiply for tile t."""
                r0 = t * T
                # fT[s, c, i]: plain strided DMA (1 KB runs per (s, c)) in 4
                # channel-slabs so the multiply can start on the first slab;
                # alternate queues to keep both DGE streams busy
                fT = fpool.tile([P, C, T], F16, tag="fT")
                f_eng = nc.sync if t % 2 == 0 else nc.scalar
                f_slab = f_d[t * S : (t + 1) * S, :].rearrange(
                    "s (c i) -> s c i", c=C
                )
                nsplit = 4 if t < 2 else 1
                for q in range(nsplit):
                    c0 = q * (C // nsplit)
                    f_eng.dma_start(
                        out=fT[:, c0 : c0 + C // nsplit, :],
                        in_=f_slab[:, c0 : c0 + C // nsplit, :],
                    )
                rhoT = rho_all[:, t, :]

                # cumsum over s (partition axis) via triangular matmuls
                pexc = psc.tile([P, T], F32, tag="pexc")
                pinc = psc.tile([P, T], F32, tag="pinc")
                nc.tensor.matmul(pexc, u_excl, rhoT, start=True, stop=True)
                nc.tensor.matmul(pinc, u_incl, rhoT, start=True, stop=True)

                # exps in fp32 (w = e1 - e2 cancels; fp16 here costs ~4% on w)
                e1 = spool.tile([P, T], F32, tag="e1")
                e2 = spool.tile([P, T], F32, tag="e2")
                nc.scalar.activation(
                    e1, pexc, mybir.ActivationFunctionType.Exp, scale=-DT
                )
                nc.scalar.activation(
                    e2, pinc, mybir.ActivationFunctionType.Exp, scale=-DT
                )
                w = spool.tile([P, T], F16, tag="w")
                nc.vector.tensor_sub(w, e1, e2)

                # tmp[s, c, i] = fT[s, c, i] * w[s, i], per loaded slab
                tmp = tpool.tile([P, C, T], F16, tag="tmp")
                for q in range(nsplit):
                    c0 = q * (C // nsplit)
                    nc.vector.tensor_mul(
                        tmp[:, c0 : c0 + C // nsplit, :],
                        fT[:, c0 : c0 + C // nsplit, :],
                        w[:, None, :].broadcast_to((P, C // nsplit, T)),
                    )
                return tmp

            def tile_back(t, tmp_pair):
                """Reduce + evacuate for a pair of tiles, sharing each E_c
                weight load across both tiles' matmuls."""
                psums = [pso.tile([C, T], F32, tag=f"po{i}", name=f"po{i}_{t}") for i in range(2)]
                for c in range(C):
                    lhs = consts[:, e_base + c * C : e_base + (c + 1) * C]
                    for tmp_t, po in zip(tmp_pair, psums):
                        nc.tensor.matmul(
                            po,
                            lhs,
                            tmp_t[:, c, :],
                            start=(c == 0),
                            stop=(c == C - 1),
                        )
                out_pair = opool.tile([C, 2 * T], F32, tag="out_pair",
                                      name=f"out_pair_{t}")
                for k, po in enumerate(psums):
                    nc.scalar.activation(
                        out_pair[:, k * T : (k + 1) * T],
                        po,
                        mybir.ActivationFunctionType.Copy,
                    )
                # stream this pair's output out now (overlaps later tiles)
                nc.scalar.dma_start(
                    out=out_d[:, t * T : (t + 2) * T],
                    in_=out_pair,
                )

            for t in range(0, ntiles, 2):
                tmp_a = tile_front(t)
                tmp_b = tile_front(t + 1)
                tile_back(t, [tmp_a, tmp_b])
    if not nc.is_finalized():
        nc.finalize()
    return nc


def _consts() -> np.ndarray:
    u_excl = np.triu(np.ones((P, P), np.float16), 1)
    u_incl = np.triu(np.ones((P, P), np.float16), 0)
    # E[:, c*C + m] = 1 if m == c else 0  (all rows identical)
    e = np.tile(np.eye(C, dtype=np.float16).reshape(1, C * C), (P, 1))
    return np.ascontiguousarray(np.concatenate([u_excl, u_incl, e], axis=1))


def kernel(rho: np.ndarray, f: np.ndarray) -> np.ndarray:
    global LAST_RESULTS
    if "nc" not in _cached:
        _cached["nc"] = _build_nc()
        _cached["consts"] = _consts()
    nc = _cached["nc"]

    rho16 = np.asarray(rho, dtype=np.float16).reshape(N, S)
    f16 = np.asarray(f, dtype=np.float16).reshape(N, S, C)
    cst = _cached["consts"]
    ntiles = NS // T

    in_maps = []
    for i in range(NCORES):
        sl = slice(i * NS, (i + 1) * NS)
        # [rays, S] -> [ntiles, S, T] / [rays, S, C] -> [ntiles, S, C, T]
        rho_t = np.ascontiguousarray(
            rho16[sl].reshape(ntiles, T, S).transpose(0, 2, 1)
        ).reshape(ntiles * S, T)
        f_t = np.ascontiguousarray(
            f16[sl].reshape(ntiles, T, S, C).transpose(0, 2, 3, 1)
        ).reshape(ntiles * S, C * T)
        in_maps.append({"rho": rho_t, "f": f_t, "consts": cst})
    res = run_bass_kernel_spmd(nc, in_maps, list(range(NCORES)), trace=TRACE)
    LAST_RESULTS = res
    out = np.concatenate(
        [res.results[i]["out"] for i in range(NCORES)], axis=1
    )  # [C, N]
    return out.reshape(C, H, W)[None].astype(np.float32, copy=False)


# revision 42
# speedup vs baseline: 1.2807x; 1.0841x over previous
"""Trainium2 Bass kernel for nn_DifferentiableProjector (volume rendering).

Math (per ray i, samples s=0..S-1, channels c):
    T_excl[s] = exp(-DT * sum_{s'<s} rho[s'])
    T_incl[s] = exp(-DT * sum_{s'<=s} rho[s'])
    w[s]      = T_excl[s] - T_incl[s]        (= T_excl * alpha)
    out[i,c]  = sum_s w[s] * f[i,s,c]

Sharding: data-parallel over rays, 65536 rays -> 8 cores x 8192 rays.

Design (all compute in "transposed space", s on partitions):
  - host casts rho/f to fp16 AND pre-tiles them so each 512-ray tile is a
    fully contiguous [S, C, T] DRAM slab (16 KB/partition rows) -> plain
    max-bandwidth DMAs, alternating the two HWDGE queues (sync/scalar)
  - cumsum over s (the partition axis) via triangular-ones matmuls on
    TensorE (fp32 PSUM); w = exp(-DT*excl) - exp(-DT*incl) with fp32 exps
    (fp16 exps would cancel catastrophically), cast to fp16
  - the big multiply runs on VectorE at 2x (fp16; the w broadcast is over
    the OUTER free dim so the innermost step stays 1)
  - segment-reduce over s: 16 accumulating one-hot matmuls on TensorE
    route channel-c column sums into PSUM row c -> [16, T] per tile;
    E_c weight loads are shared across tile pairs
  - ScalarE evacuates PSUM -> SBUF; outputs stream out per tile-pair,
    c-major [C, rays] = exactly the final [1,C,H,W] layout

Measured on 8 axon trn2 cores: ~129 us HW exec (v1 naive DVE: 415 us);
fp16 DMA roofline ~96 us + ~22 us fixed startup/tail. Rel err ~4e-4.
"""

import numpy as np

import concourse.bass as bass
import concourse.tile as tile
from concourse.bacc import Bacc
from concourse import mybir
from concourse.bass_utils import run_bass_kernel_spmd

H, W, S, C = 256, 256, 128, 16
N = H * W
NCORES = 8
NS = N // NCORES          # rays per core
P = 128                   # partitions (= S)
T = 512                   # rays per tile
DT = (6.0 - 2.0) / S

_cached = {}

# test-harness hooks (ignored by grading path)
TRACE = False
LAST_RESULTS = None

F16 = mybir.dt.float16
F32 = mybir.dt.float32


def _build_nc(ns: int = NS) -> bass.Bass:
    ntiles = ns // T
    nc = Bacc()
    # host supplies tensors pre-transposed AND pre-tiled:
    #   rho [ntiles*S, T]  (tile t rows t*S:(t+1)*S = rho[s, i] slab)
    #   f   [ntiles*S, C*T] (tile t rows = f[s, (c, i)] slab, contiguous)
    rho_d = nc.dram_tensor("rho", [ntiles * S, T], F16, kind="ExternalInput")
    f_d = nc.dram_tensor("f", [ntiles * S, C * T], F16, kind="ExternalInput")
    cst_d = nc.dram_tensor("consts", [P, 2 * P + C * C], F16, kind="ExternalInput")
    out_d = nc.dram_tensor("out", [C, ns], F32, kind="ExternalOutput")

    with tile.TileContext(nc) as tc:
        with (
            tc.tile_pool(name="cpool", bufs=1) as cpool,
            tc.tile_pool(name="fpool", bufs=6) as fpool,
            tc.tile_pool(name="tpool", bufs=3) as tpool,
            tc.tile_pool(name="spool", bufs=4) as spool,
            tc.tile_pool(name="opool", bufs=3) as opool,
            tc.tile_pool(name="psc", bufs=2, space="PSUM") as psc,
            tc.tile_pool(name="pso", bufs=2, space="PSUM") as pso,
        ):
            consts = cpool.tile([P, 2 * P + C * C], F16)
            nc.scalar.dma_start(out=consts, in_=cst_d[:, :])
            u_excl = consts[:, 0:P]
            u_incl = consts[:, P : 2 * P]
            # E_c = consts[:, 2P + 16c : 2P + 16c + 16]: column m one-hot at c
            e_base = 2 * P


            # all rho slabs upfront: [s, (tile, i)] fp16 = 16 KB/partition
            rho_all = cpool.tile([P, ntiles, T], F16)
            nc.sync.dma_start(
                out=rho_all,
                in_=rho_d[:, :].rearrange("(t s) i -> s t i", s=S),
            )

            def tile_front(t):
                """DMA loads + w pipeline + big multiply for tile t."""
                r0 = t * T
                # fT[s, c, i]: contiguous 16 KB/partition slab DMA; the first
                # two tiles split 4-way so the multiply starts sooner;
                # alternate queues to keep both DGE streams busy
                fT = fpool.tile([P, C, T], F16, tag="fT")
                f_eng = nc.sync if t % 2 == 0 else nc.scalar
                f_slab = f_d[t * S : (t + 1) * S, :].rearrange(
                    "s (c i) -> s c i", c=C
                )
                nsplit = 4 if t < 2 else 2
                for q in range(nsplit):
                    c0 = q * (C // nsplit)
                    f_eng.dma_start(
                        out=fT[:, c0 : c0 + C // nsplit, :],
                        in_=f_slab[:, c0 : c0 + C // nsplit, :],
                    )
                rhoT = rho_all[:, t, :]

                # cumsum over s (partition axis) via triangular matmuls
                pexc = psc.tile([P, T], F32, tag="pexc")
                pinc = psc.tile([P, T], F32, tag="pinc")
                nc.tensor.matmul(pexc, u_excl, rhoT, start=True, stop=True)
                nc.tensor.matmul(pinc, u_incl, rhoT, start=True, stop=True)

                # exps in fp32 (w = e1 - e2 cancels; fp16 here costs ~4% on w)
                e1 = spool.tile([P, T], F32, tag="e1")
                e2 = spool.tile([P, T], F32, tag="e2")
                nc.scalar.activation(
                    e1, pexc, mybir.ActivationFunctionType.Exp, scale=-DT
                )
                nc.scalar.activation(
                    e2, pinc, mybir.ActivationFunctionType.Exp, scale=-DT
                )
                w = spool.tile([P, T], F16, tag="w")
                nc.vector.tensor_sub(w, e1, e2)

                # tmp[s, c, i] = fT[s, c, i] * w[s, i], per loaded slab
                tmp = tpool.tile([P, C, T], F16, tag="tmp")
                for q in range(nsplit):
                    c0 = q * (C // nsplit)
                    nc.vector.tensor_mul(
                        tmp[:, c0 : c0 + C // nsplit, :],
                        fT[:, c0 : c0 + C // nsplit, :],
                        w[:, None, :].broadcast_to((P, C // nsplit, T)),
                    )
                return tmp

            def tile_back(t, tmp_pair):
                """Reduce + evacuate for a pair of tiles, sharing each E_c
                weight load across both tiles' matmuls."""
                psums = [pso.tile([C, T], F32, tag=f"po{i}", name=f"po{i}_{t}") for i in range(2)]
                for c in range(C):
                    lhs = consts[:, e_base + c * C : e_base + (c + 1) * C]
                    for tmp_t, po in zip(tmp_pair, psums):
                        nc.tensor.matmul(
                            po,
                            lhs,
                            tmp_t[:, c, :],
                            start=(c == 0),
                            stop=(c == C - 1),
                        )
                out_pair = opool.tile([C, 2 * T], F32, tag="out_pair",
                                      name=f"out_pair_{t}")
                for k, po in enumerate(psums):
                    nc.scalar.activation(
                        out_pair[:, k * T : (k + 1) * T],
                        po,
                        mybir.ActivationFunctionType.Copy,
                    )
                # stream this pair's output out now (overlaps later tiles)
                nc.scalar.dma_start(
                    out=out_d[:, t * T : (t + 2) * T],
                    in_=out_pair,
                )

            for t in range(0, ntiles, 2):
                tmp_a = tile_front(t)
                tmp_b = tile_front(t + 1)
                tile_back(t, [tmp_a, tmp_b])
    if not nc.is_finalized():
        nc.finalize()
    return nc


def _consts() -> np.ndarray:
    u_excl = np.triu(np.ones((P, P), np.float16), 1)
    u_incl = np.triu(np.ones((P, P), np.float16), 0)
    # E[:, c*C + m] = 1 if m == c else 0  (all rows identical)
    e = np.tile(np.eye(C, dtype=np.float16).reshape(1, C * C), (P, 1))
    return np.ascontiguousarray(np.concatenate([u_excl, u_incl, e], axis=1))


def kernel(rho: np.ndarray, f: np.ndarray) -> np.ndarray:
    global LAST_RESULTS
    if "nc" not in _cached:
        _cached["nc"] = _build_nc()
        _cached["consts"] = _consts()
    nc = _cached["nc"]

    rho16 = np.asarray(rho, dtype=np.float16).reshape(N, S)
    f16 = np.asarray(f, dtype=np.float16).reshape(N, S, C)
    cst = _cached["consts"]
    ntiles = NS // T

    in_maps = []
    for i in range(NCORES):
        sl = slice(i * NS, (i + 1) * NS)
        # [rays, S] -> [ntiles, S, T] / [rays, S, C] -> [ntiles, S, C, T]
        rho_t = np.ascontiguousarray(
            rho16[sl].reshape(ntiles, T, S).transpose(0, 2, 1)
        ).reshape(ntiles * S, T)
        f_t = np.ascontiguousarray(
            f16[sl].reshape(ntiles, T, S, C).transpose(0, 2, 3, 1)
        ).reshape(ntiles * S, C * T)
        in_maps.append({"rho": rho_t, "f": f_t, "consts": cst})
    res = run_bass_kernel_spmd(nc, in_maps, list(range(NCORES)), trace=TRACE)
    LAST_RESULTS = res
    out = np.concatenate(
        [res.results[i]["out"] for i in range(NCORES)], axis=1
    )  # [C, N]
    return out.reshape(C, H, W)[None].astype(np.float32, copy=False)


# revision 43
# speedup vs baseline: 1.2891x; 1.0066x over previous
"""Trainium2 Bass kernel for nn_DifferentiableProjector (volume rendering).

Math (per ray i, samples s=0..S-1, channels c):
    T_excl[s] = exp(-DT * sum_{s'<s} rho[s'])
    T_incl[s] = exp(-DT * sum_{s'<=s} rho[s'])
    w[s]      = T_excl[s] - T_incl[s]        (= T_excl * alpha)
    out[i,c]  = sum_s w[s] * f[i,s,c]

Sharding: data-parallel over rays, 65536 rays -> 8 cores x 8192 rays.

Design (all compute in "transposed space", s on partitions):
  - host casts rho/f to fp16 AND pre-tiles them so each 512-ray tile is a
    fully contiguous [S, C, T] DRAM slab (16 KB/partition rows) -> plain
    max-bandwidth DMAs, alternating the two HWDGE queues (sync/scalar)
  - cumsum over s (the partition axis) via triangular-ones matmuls on
    TensorE (fp32 PSUM); w = exp(-DT*excl) - exp(-DT*incl) with fp32 exps
    (fp16 exps would cancel catastrophically), cast to fp16
  - the big multiply runs on VectorE at 2x (fp16; the w broadcast is over
    the OUTER free dim so the innermost step stays 1)
  - segment-reduce over s: 16 accumulating one-hot matmuls on TensorE
    route channel-c column sums into PSUM row c -> [16, T] per tile;
    E_c weight loads are shared across tile pairs
  - ScalarE evacuates PSUM -> SBUF; outputs stream out per tile-pair,
    c-major [C, rays] = exactly the final [1,C,H,W] layout

Measured on 8 axon trn2 cores: ~129 us HW exec (v1 naive DVE: 415 us);
fp16 DMA roofline ~96 us + ~22 us fixed startup/tail. Rel err ~4e-4.
"""

import numpy as np

import concourse.bass as bass
import concourse.tile as tile
from concourse.bacc import Bacc
from concourse import mybir
from concourse.bass_utils import run_bass_kernel_spmd

H, W, S, C = 256, 256, 128, 16
N = H * W
NCORES = 8
NS = N // NCORES          # rays per core
P = 128                   # partitions (= S)
T = 512                   # rays per tile
DT = (6.0 - 2.0) / S

_cached = {}

# test-harness hooks (ignored by grading path)
TRACE = False
LAST_RESULTS = None

F16 = mybir.dt.float16
F32 = mybir.dt.float32


def _build_nc(ns: int = NS) -> bass.Bass:
    ntiles = ns // T
    nc = Bacc()
    # host supplies tensors pre-transposed AND pre-tiled:
    #   rho [ntiles*S, T]  (tile t rows t*S:(t+1)*S = rho[s, i] slab)
    #   f   [ntiles*S, C*T] (tile t rows = f[s, (c, i)] slab, contiguous)
    rho_d = nc.dram_tensor("rho", [ntiles * S, T], F16, kind="ExternalInput")
    f_d = nc.dram_tensor("f", [ntiles * S, C * T], F16, kind="ExternalInput")
    cst_d = nc.dram_tensor("consts", [P, 2 * P + C * C], F16, kind="ExternalInput")
    out_d = nc.dram_tensor("out", [C, ns], F32, kind="ExternalOutput")

    with tile.TileContext(nc) as tc:
        with (
            tc.tile_pool(name="cpool", bufs=1) as cpool,
            tc.tile_pool(name="fpool", bufs=6) as fpool,
            tc.tile_pool(name="tpool", bufs=3) as tpool,
            tc.tile_pool(name="spool", bufs=4) as spool,
            tc.tile_pool(name="opool", bufs=3) as opool,
            tc.tile_pool(name="psc", bufs=2, space="PSUM") as psc,
            tc.tile_pool(name="pso", bufs=2, space="PSUM") as pso,
        ):
            consts = cpool.tile([P, 2 * P + C * C], F16)
            nc.scalar.dma_start(out=consts, in_=cst_d[:, :])
            u_excl = consts[:, 0:P]
            u_incl = consts[:, P : 2 * P]
            # E_c = consts[:, 2P + 16c : 2P + 16c + 16]: column m one-hot at c
            e_base = 2 * P


            # all rho slabs upfront: [s, (tile, i)] fp16 = 16 KB/partition
            rho_all = cpool.tile([P, ntiles, T], F16)
            nc.sync.dma_start(
                out=rho_all,
                in_=rho_d[:, :].rearrange("(t s) i -> s t i", s=S),
            )

            def tile_front(t):
                """DMA loads + w pipeline + big multiply for tile t."""
                r0 = t * T
                # fT[s, c, i]: contiguous 16 KB/partition slab DMA; the first
                # two tiles split 4-way so the multiply starts sooner;
                # alternate queues to keep both DGE streams busy
                fT = fpool.tile([P, C, T], F16, tag="fT")
                f_eng = nc.sync if t % 2 == 0 else nc.scalar
                f_slab = f_d[t * S : (t + 1) * S, :].rearrange(
                    "s (c i) -> s c i", c=C
                )
                nsplit = 4
                for q in range(nsplit):
                    c0 = q * (C // nsplit)
                    f_eng.dma_start(
                        out=fT[:, c0 : c0 + C // nsplit, :],
                        in_=f_slab[:, c0 : c0 + C // nsplit, :],
                    )
                rhoT = rho_all[:, t, :]

                # cumsum over s (partition axis) via triangular matmuls
                pexc = psc.tile([P, T], F32, tag="pexc")
                pinc = psc.tile([P, T], F32, tag="pinc")
                nc.tensor.matmul(pexc, u_excl, rhoT, start=True, stop=True)
                nc.tensor.matmul(pinc, u_incl, rhoT, start=True, stop=True)

                # exps in fp32 (w = e1 - e2 cancels; fp16 here costs ~4% on w)
                e1 = spool.tile([P, T], F32, tag="e1")
                e2 = spool.tile([P, T], F32, tag="e2")
                nc.scalar.activation(
                    e1, pexc, mybir.ActivationFunctionType.Exp, scale=-DT
                )
                nc.scalar.activation(
                    e2, pinc, mybir.ActivationFunctionType.Exp, scale=-DT
                )
                w = spool.tile([P, T], F16, tag="w")
                nc.vector.tensor_sub(w, e1, e2)

                # tmp[s, c, i] = fT[s, c, i] * w[s, i], per loaded slab
                tmp = tpool.tile([P, C, T], F16, tag="tmp")
                for q in range(nsplit):
                    c0 = q * (C // nsplit)
                    nc.vector.tensor_mul(
                        tmp[:, c0 : c0 + C // nsplit, :],
                        fT[:, c0 : c0 + C // nsplit, :],
                        w[:, None, :].broadcast_to((P, C // nsplit, T)),
                    )
                return tmp

            def tile_back(t, tmp_pair):
                """Reduce + evacuate for a pair of tiles, sharing each E_c
                weight load across both tiles' matmuls."""
                psums = [pso.tile([C, T], F32, tag=f"po{i}", name=f"po{i}_{t}") for i in range(2)]
                for c in range(C):
                    lhs = consts[:, e_base + c * C : e_base + (c + 1) * C]
                    for tmp_t, po in zip(tmp_pair, psums):
                        nc.tensor.matmul(
                            po,
                            lhs,
                            tmp_t[:, c, :],
                            start=(c == 0),
                            stop=(c == C - 1),
                        )
                out_pair = opool.tile([C, 2 * T], F32, tag="out_pair",
                                      name=f"out_pair_{t}")
                for k, po in enumerate(psums):
                    nc.scalar.activation(
                        out_pair[:, k * T : (k + 1) * T],
                        po,
                        mybir.ActivationFunctionType.Copy,
                    )
                # stream this pair's output out now (overlaps later tiles)
                nc.scalar.dma_start(
                    out=out_d[:, t * T : (t + 2) * T],
                    in_=out_pair,
                )

            for t in range(0, ntiles, 2):
                tmp_a = tile_front(t)
                tmp_b = tile_front(t + 1)
                tile_back(t, [tmp_a, tmp_b])
    if not nc.is_finalized():
        nc.finalize()
    return nc


def _consts() -> np.ndarray:
    u_excl = np.triu(np.ones((P, P), np.float16), 1)
    u_incl = np.triu(np.ones((P, P), np.float16), 0)
    # E[:, c*C + m] = 1 if m == c else 0  (all rows identical)
    e = np.tile(np.eye(C, dtype=np.float16).reshape(1, C * C), (P, 1))
    return np.ascontiguousarray(np.concatenate([u_excl, u_incl, e], axis=1))


def kernel(rho: np.ndarray, f: np.ndarray) -> np.ndarray:
    global LAST_RESULTS
    if "nc" not in _cached:
        _cached["nc"] = _build_nc()
        _cached["consts"] = _consts()
    nc = _cached["nc"]

    rho16 = np.asarray(rho, dtype=np.float16).reshape(N, S)
    f16 = np.asarray(f, dtype=np.float16).reshape(N, S, C)
    cst = _cached["consts"]
    ntiles = NS // T

    in_maps = []
    for i in range(NCORES):
        sl = slice(i * NS, (i + 1) * NS)
        # [rays, S] -> [ntiles, S, T] / [rays, S, C] -> [ntiles, S, C, T]
        rho_t = np.ascontiguousarray(
            rho16[sl].reshape(ntiles, T, S).transpose(0, 2, 1)
        ).reshape(ntiles * S, T)
        f_t = np.ascontiguousarray(
            f16[sl].reshape(ntiles, T, S, C).transpose(0, 2, 3, 1)
        ).reshape(ntiles * S, C * T)
        in_maps.append({"rho": rho_t, "f": f_t, "consts": cst})
    res = run_bass_kernel_spmd(nc, in_maps, list(range(NCORES)), trace=TRACE)
    LAST_RESULTS = res
    out = np.concatenate(
        [res.results[i]["out"] for i in range(NCORES)], axis=1
    )  # [C, N]
    return out.reshape(C, H, W)[None].astype(np.float32, copy=False)
